# revision 3
# baseline (speedup 1.0000x reference)
"""Trainium2 Bass kernel for nn_Model_1331439862418.

4-layer stacked tanh-RNN with ReLU+AvgPool1d(k=7,s=5) between layers, final FC.
Data-parallel: B=512 sharded over 8 cores (64 batch each); each core runs the
full sequential scan chain.

Per-core design (all layers pipelined at step granularity):
  - layer-1 xproj: K=1 outer-product matmul from a DMA-streamed flat x.T buffer
  - layer>=2 xproj: ReLU+avgpool+input-projection fused into 7 accumulating
    "tap" matmuls (W_ih.T/7 @ relu_ring_slot) into the step's PSUM tile
  - recurrence: one matmul W_hh.T @ h_prev accumulated into the same PSUM bank
  - activation: tanh(psum + per-partition bias) on ScalarE -> h ring
  - relu: tensor_scalar_max on VectorE -> r ring (feeds next layer's taps)
  - FC: 35 accumulating taps (fc_w.T/7 slices @ r4 slots) + bias add, at tail

kernel(**inputs) takes FULL unsharded inputs, returns FULL [512, 10] output.
"""

import numpy as np

import concourse.bass as bass  # noqa: F401  (bass types used via bacc/tile)
import concourse.mybir as mybir
import concourse.tile as tile
from concourse import bacc
from concourse.bass_utils import run_bass_kernel_spmd

F32 = mybir.dt.float32
AF = mybir.ActivationFunctionType

NCORES = 8
B = 64          # batch per core
POOL_K, POOL_S = 7, 5
HS = [16, 32, 64, 128]
IS = [1, 16, 32, 64]

XCH = 64        # x-stream chunk length (steps)
XSLOTS = 4      # x-stream ring slots
RH = 8          # h ring slots per layer
MARGIN = 2      # parent steps between window-complete and child step emission


def seq_lens(T0):
    T = [T0]
    for _ in range(3):
        T.append((T[-1] - POOL_K) // POOL_S + 1)
    W4 = (T[3] - POOL_K) // POOL_S + 1
    return T, W4


def build(T0):
    """Build + compile the per-core Bass program. Returns compiled nc."""
    T, W4 = seq_lens(T0)
    nc = bacc.Bacc("TRN2", target_bir_lowering=False, debug=False,
                   num_devices=NCORES, enable_asserts=False)

    xq_d = nc.dram_tensor("xq", [1, T0 * B], F32, kind="ExternalInput")
    wih_d = [nc.dram_tensor(f"wih{l}", [IS[l], HS[l]], F32, kind="ExternalInput")
             for l in range(4)]
    whh_d = [nc.dram_tensor(f"whh{l}", [HS[l], HS[l]], F32, kind="ExternalInput")
             for l in range(4)]
    b_d = [nc.dram_tensor(f"b{l}", [HS[l], 1], F32, kind="ExternalInput")
           for l in range(4)]
    fcw_d = nc.dram_tensor("fcw", [W4 * 128, 10], F32, kind="ExternalInput")
    fcb_d = nc.dram_tensor("fcb", [10, 1], F32, kind="ExternalInput")
    out_d = nc.dram_tensor("out", [10, B], F32, kind="ExternalOutput")

    RR = [32, 32, 32, T[3]]     # relu ring slots per layer (r4 holds all steps)

    with tile.TileContext(nc) as tc:
        with (
            tc.tile_pool(name="const", bufs=1) as constp,
            tc.tile_pool(name="ring", bufs=1) as ringp,
            tc.tile_pool(name="ps1", bufs=2, space="PSUM") as ps1,
            tc.tile_pool(name="ps2", bufs=2, space="PSUM") as ps2,
            tc.tile_pool(name="ps3", bufs=2, space="PSUM") as ps3,
            tc.tile_pool(name="ps4", bufs=2, space="PSUM") as ps4,
        ):
            psp = [ps1, ps2, ps3, ps4]

            wih, whh, bias = [], [], []
            for l in range(4):
                w1 = constp.tile([IS[l], HS[l]], F32, tag=f"wih{l}")
                nc.sync.dma_start(out=w1, in_=wih_d[l].ap())
                wih.append(w1)
                w2 = constp.tile([HS[l], HS[l]], F32, tag=f"whh{l}")
                nc.sync.dma_start(out=w2, in_=whh_d[l].ap())
                whh.append(w2)
                bb = constp.tile([HS[l], 1], F32, tag=f"b{l}")
                nc.sync.dma_start(out=bb, in_=b_d[l].ap())
                bias.append(bb)
            fc_sb = constp.tile([128, W4, 10], F32, tag="fcw")
            nc.sync.dma_start(out=fc_sb,
                              in_=fcw_d.ap().rearrange("(j p) o -> p j o", p=128))
            fcb_sb = constp.tile([10, 1], F32, tag="fcb")
            nc.sync.dma_start(out=fcb_sb, in_=fcb_d.ap())

            xq = ringp.tile([1, XSLOTS * XCH * B], F32, tag="xq")
            h = [ringp.tile([HS[l], RH * B], F32, tag=f"h{l}", name=f"h{l}") for l in range(4)]
            r = [ringp.tile([HS[l], RR[l] * B], F32, tag=f"r{l}", name=f"r{l}") for l in range(4)]

            nchunks = (T0 + XCH - 1) // XCH

            def emit_xq_dma(c):
                if c >= nchunks:
                    return
                n = min(XCH, T0 - c * XCH) * B
                base = (c % XSLOTS) * XCH * B
                nc.sync.dma_start(out=xq[0:1, base:base + n],
                                  in_=xq_d.ap()[0:1, c * XCH * B:c * XCH * B + n])

            pswin = [dict() for _ in range(4)]   # layer -> window j -> psum tile
            ready = [None, [], [], []]           # ready-to-emit child windows

            def emit_tap(l, j, k):
                s = POOL_S * j + k               # parent-layer step index
                if k == 0:
                    pswin[l][j] = psp[l].tile([HS[l], B], F32, tag=f"ps{l}", name=f"psw{l}")
                ps = pswin[l][j]
                slot = s % RR[l - 1]
                nc.tensor.matmul(
                    ps, lhsT=wih[l], rhs=r[l - 1][:, slot * B:(slot + 1) * B],
                    start=(k == 0), stop=(k == POOL_K - 1 and j == 0),
                    skip_group_check=True)

            def emit_step(l, t):
                if l == 0:
                    ps = psp[0].tile([HS[0], B], F32, tag="ps0", name="ps0t")
                    off = ((t // XCH) % XSLOTS) * XCH * B + (t % XCH) * B
                    nc.tensor.matmul(ps, lhsT=wih[0], rhs=xq[0:1, off:off + B],
                                     start=True, stop=(t == 0),
                                     skip_group_check=True)
                else:
                    ps = pswin[l].pop(t)
                if t > 0:
                    hp = (t - 1) % RH
                    nc.tensor.matmul(ps, lhsT=whh[l],
                                     rhs=h[l][:, hp * B:(hp + 1) * B],
                                     start=False, stop=True,
                                     skip_group_check=True)
                hc = t % RH
                nc.scalar.activation(out=h[l][:, hc * B:(hc + 1) * B], in_=ps,
                                     func=AF.Tanh, bias=bias[l][:, 0:1], scale=1.0)
                rs = t % RR[l]
                nc.vector.tensor_scalar_max(r[l][:, rs * B:(rs + 1) * B],
                                            h[l][:, hc * B:(hc + 1) * B], 0.0)
                after_step(l, t)

            def after_step(l, s):
                if l == 3:
                    return                       # FC handled at tail
                c = l + 1
                n_child = T[c]
                jlo = max(0, -(-(s - (POOL_K - 1)) // POOL_S))  # ceil((s-6)/5)
                jhi = min(n_child - 1, s // POOL_S)
                for j in range(jlo, jhi + 1):
                    emit_tap(c, j, s - POOL_S * j)
                    if s - POOL_S * j == POOL_K - 1:
                        ready[c].append(j)
                while ready[c] and POOL_S * ready[c][0] + POOL_K - 1 + MARGIN <= s:
                    emit_step(c, ready[c].pop(0))

            # ---- main pipeline ----
            for c in range(min(XSLOTS - 1, nchunks)):
                emit_xq_dma(c)
            for t in range(T0):
                if t % XCH == 0:
                    emit_xq_dma(t // XCH + XSLOTS - 1)
                emit_step(0, t)
            for l in (1, 2, 3):                  # tail flush
                while ready[l]:
                    emit_step(l, ready[l].pop(0))

            # ---- FC tail ----
            ps_fc = psp[0].tile([10, B], F32, tag="ps0", name="psfc")
            for j in range(W4):
                for k in range(POOL_K):
                    s = POOL_S * j + k
                    nc.tensor.matmul(ps_fc, lhsT=fc_sb[:, j, :],
                                     rhs=r[3][:, s * B:(s + 1) * B],
                                     start=(j == 0 and k == 0),
                                     stop=(j == W4 - 1 and k == POOL_K - 1),
                                     skip_group_check=True)
            out_sb = constp.tile([10, B], F32, tag="out_sb")
            nc.vector.tensor_scalar_add(out_sb, ps_fc, fcb_sb[:, 0:1])
            nc.sync.dma_start(out=out_d.ap(), in_=out_sb)

    nc.compile()
    return nc


def prep_in_maps(inputs, T0):
    """Host-side prep: shard x, transpose/scale weights. Returns per-core maps."""
    T, W4 = seq_lens(T0)
    f = lambda a: np.ascontiguousarray(np.asarray(a, dtype=np.float32))
    x = f(inputs["x"]).reshape(-1, T0)          # [512, T0]
    nb = x.shape[0] // B

    common = {}
    for l in range(4):
        wi = f(inputs[f"w_ih{l + 1}"])          # [H, I]
        wh = f(inputs[f"w_hh{l + 1}"])          # [H, H]
        bi = f(inputs[f"b_ih{l + 1}"]) + f(inputs[f"b_hh{l + 1}"])
        scale = 1.0 if l == 0 else (1.0 / POOL_K)
        common[f"wih{l}"] = np.ascontiguousarray((wi * scale).T)   # [I, H]
        common[f"whh{l}"] = np.ascontiguousarray(wh.T)             # [H, H]
        common[f"b{l}"] = np.ascontiguousarray(bi.reshape(-1, 1))  # [H, 1]
    common["fcw"] = np.ascontiguousarray((f(inputs["fc_w"]) / POOL_K).T)  # [640,10]
    common["fcb"] = np.ascontiguousarray(f(inputs["fc_b"]).reshape(-1, 1))

    in_maps = []
    for c in range(nb):
        m = dict(common)
        xc = x[c * B:(c + 1) * B]               # [B, T0]
        m["xq"] = np.ascontiguousarray(xc.T).reshape(1, T0 * B)
        in_maps.append(m)
    return in_maps


_NC_CACHE = {}


def _install_ntff_hook():
    """Register the axon NTFF profile hook (the agent image's antenv lacks
    axon_hooks, so run_bass_kernel_spmd's trace path can't find it)."""
    import sys
    import types
    if "antenv.axon_hooks" in sys.modules:
        return
    mod = types.ModuleType("antenv.axon_hooks")
    mod._hook = None
    mod.set_axon_ntff_profile_hook = lambda h: setattr(mod, "_hook", h)
    mod.get_axon_ntff_profile_hook = lambda: mod._hook
    sys.modules["antenv.axon_hooks"] = mod
    try:
        import antenv
        antenv.axon_hooks = mod
    except ImportError:
        pass
    try:
        from trn_agent_boot.trn_boot import _ntff_profile_via_ctypes
        mod._hook = _ntff_profile_via_ctypes("/opt/axon/libaxon_pjrt.so")
    except Exception as e:  # degrade to no tracing
        print("ntff hook install failed:", e)


def run(inputs, T0=3437, core_ids=None, trace=False):
    if trace:
        _install_ntff_hook()
    if T0 not in _NC_CACHE:
        _NC_CACHE[T0] = build(T0)
    nc = _NC_CACHE[T0]
    in_maps = prep_in_maps(inputs, T0)
    if core_ids is None:
        core_ids = list(range(len(in_maps)))
    res = run_bass_kernel_spmd(nc, in_maps, core_ids=core_ids, trace=trace)
    out = np.concatenate([res.results[i]["out"].T for i in range(len(in_maps))],
                         axis=0).astype(np.float32)
    return out, res


def kernel(**inputs) -> np.ndarray:
    out, _ = run(inputs)
    return out


# revision 4
# speedup vs baseline: 1.5614x; 1.5614x over previous
"""Trainium2 Bass kernel for nn_Model_1331439862418.

4-layer stacked tanh-RNN with ReLU+AvgPool1d(k=7,s=5) between layers, final FC.
Data-parallel: B=512 sharded over 8 cores (64 batch each); each core runs the
full sequential scan chain.

Per-core design (all layers pipelined at step granularity):
  - layer-1 xproj: K=1 outer-product matmul from a DMA-streamed flat x.T buffer
  - layer>=2 xproj: ReLU+avgpool+input-projection fused into 7 accumulating
    "tap" matmuls (W_ih.T/7 @ relu_ring_slot) into the step's PSUM tile
  - recurrence: one matmul W_hh.T @ h_prev accumulated into the same PSUM bank
  - activation: tanh(psum + per-partition bias) on ScalarE -> h ring
  - relu: tensor_scalar_max on VectorE -> r ring (feeds next layer's taps)
  - FC: 35 accumulating taps (fc_w.T/7 slices @ r4 slots) + bias add, at tail

kernel(**inputs) takes FULL unsharded inputs, returns FULL [512, 10] output.
"""

import numpy as np

import concourse.bass as bass  # noqa: F401  (bass types used via bacc/tile)
import concourse.mybir as mybir
import concourse.tile as tile
from concourse import bacc
from concourse.bass_utils import run_bass_kernel_spmd

F32 = mybir.dt.float32
F16 = mybir.dt.float16
AF = mybir.ActivationFunctionType

NCORES = 8
B = 64          # batch per core
POOL_K, POOL_S = 7, 5
HS = [16, 32, 64, 128]
IS = [1, 16, 32, 64]

XCH = 64        # x-stream chunk length (steps)
XSLOTS = 4      # x-stream ring slots
RH = 8          # h ring slots per layer
MARGIN = 2      # parent steps between window-complete and child step emission


def seq_lens(T0):
    T = [T0]
    for _ in range(3):
        T.append((T[-1] - POOL_K) // POOL_S + 1)
    W4 = (T[3] - POOL_K) // POOL_S + 1
    return T, W4


def build(T0):
    """Build + compile the per-core Bass program. Returns compiled nc."""
    T, W4 = seq_lens(T0)
    nc = bacc.Bacc("TRN2", target_bir_lowering=False, debug=False,
                   num_devices=NCORES, enable_asserts=False)

    xq_d = nc.dram_tensor("xq", [1, T0 * B], F16, kind="ExternalInput")
    wih_d = [nc.dram_tensor(f"wih{l}", [IS[l], HS[l]], F16, kind="ExternalInput")
             for l in range(4)]
    whh_d = [nc.dram_tensor(f"whh{l}", [HS[l], HS[l]], F16, kind="ExternalInput")
             for l in range(4)]
    b_d = [nc.dram_tensor(f"b{l}", [HS[l], 1], F32, kind="ExternalInput")
           for l in range(4)]
    fcw_d = nc.dram_tensor("fcw", [W4 * 128, 10], F16, kind="ExternalInput")
    fcb_d = nc.dram_tensor("fcb", [10, 1], F32, kind="ExternalInput")
    out_d = nc.dram_tensor("out", [10, B], F32, kind="ExternalOutput")

    RR = [32, 32, 32, T[3]]     # relu ring slots per layer (r4 holds all steps)

    with tile.TileContext(nc) as tc:
        with (
            tc.tile_pool(name="const", bufs=1) as constp,
            tc.tile_pool(name="ring", bufs=1) as ringp,
            tc.tile_pool(name="ps1", bufs=2, space="PSUM") as ps1,
            tc.tile_pool(name="ps2", bufs=2, space="PSUM") as ps2,
            tc.tile_pool(name="ps3", bufs=2, space="PSUM") as ps3,
            tc.tile_pool(name="ps4", bufs=2, space="PSUM") as ps4,
        ):
            psp = [ps1, ps2, ps3, ps4]

            wih, whh, bias = [], [], []
            for l in range(4):
                w1 = constp.tile([IS[l], HS[l]], F16, tag=f"wih{l}")
                nc.sync.dma_start(out=w1, in_=wih_d[l].ap())
                wih.append(w1)
                w2 = constp.tile([HS[l], HS[l]], F16, tag=f"whh{l}")
                nc.sync.dma_start(out=w2, in_=whh_d[l].ap())
                whh.append(w2)
                bb = constp.tile([HS[l], 1], F32, tag=f"b{l}")
                nc.sync.dma_start(out=bb, in_=b_d[l].ap())
                bias.append(bb)
            fc_sb = constp.tile([128, W4, 10], F16, tag="fcw")
            nc.sync.dma_start(out=fc_sb,
                              in_=fcw_d.ap().rearrange("(j p) o -> p j o", p=128))
            fcb_sb = constp.tile([10, 1], F32, tag="fcb")
            nc.sync.dma_start(out=fcb_sb, in_=fcb_d.ap())

            xq = ringp.tile([1, XSLOTS * XCH * B], F16, tag="xq")
            h = [ringp.tile([HS[l], RH * B], F16, tag=f"h{l}", name=f"h{l}") for l in range(4)]
            r = [ringp.tile([HS[l], RR[l] * B], F16, tag=f"r{l}", name=f"r{l}") for l in range(4)]

            nchunks = (T0 + XCH - 1) // XCH

            def emit_xq_dma(c):
                if c >= nchunks:
                    return
                n = min(XCH, T0 - c * XCH) * B
                base = (c % XSLOTS) * XCH * B
                nc.sync.dma_start(out=xq[0:1, base:base + n],
                                  in_=xq_d.ap()[0:1, c * XCH * B:c * XCH * B + n])

            pswin = [dict() for _ in range(4)]   # layer -> window j -> psum tile
            ready = [None, [], [], []]           # ready-to-emit child windows

            def emit_tap(l, j, k):
                s = POOL_S * j + k               # parent-layer step index
                if k == 0:
                    pswin[l][j] = psp[l].tile([HS[l], B], F32, tag=f"ps{l}", name=f"psw{l}")
                ps = pswin[l][j]
                slot = s % RR[l - 1]
                nc.tensor.matmul(
                    ps, lhsT=wih[l], rhs=r[l - 1][:, slot * B:(slot + 1) * B],
                    start=(k == 0), stop=(k == POOL_K - 1 and j == 0),
                    skip_group_check=True)

            def emit_step(l, t):
                if l == 0:
                    ps = psp[0].tile([HS[0], B], F32, tag="ps0", name="ps0t")
                    off = ((t // XCH) % XSLOTS) * XCH * B + (t % XCH) * B
                    nc.tensor.matmul(ps, lhsT=wih[0], rhs=xq[0:1, off:off + B],
                                     start=True, stop=(t == 0),
                                     skip_group_check=True)
                else:
                    ps = pswin[l].pop(t)
                if t > 0:
                    hp = (t - 1) % RH
                    nc.tensor.matmul(ps, lhsT=whh[l],
                                     rhs=h[l][:, hp * B:(hp + 1) * B],
                                     start=False, stop=True,
                                     skip_group_check=True)
                hc = t % RH
                nc.scalar.activation(out=h[l][:, hc * B:(hc + 1) * B], in_=ps,
                                     func=AF.Tanh, bias=bias[l][:, 0:1], scale=1.0)
                rs = t % RR[l]
                nc.vector.tensor_scalar_max(r[l][:, rs * B:(rs + 1) * B],
                                            h[l][:, hc * B:(hc + 1) * B], 0.0)
                after_step(l, t)

            def after_step(l, s):
                if l == 3:
                    return                       # FC handled at tail
                c = l + 1
                n_child = T[c]
                jlo = max(0, -(-(s - (POOL_K - 1)) // POOL_S))  # ceil((s-6)/5)
                jhi = min(n_child - 1, s // POOL_S)
                for j in range(jlo, jhi + 1):
                    emit_tap(c, j, s - POOL_S * j)
                    if s - POOL_S * j == POOL_K - 1:
                        ready[c].append(j)
                while ready[c] and POOL_S * ready[c][0] + POOL_K - 1 + MARGIN <= s:
                    emit_step(c, ready[c].pop(0))

            # ---- main pipeline ----
            for c in range(min(XSLOTS - 1, nchunks)):
                emit_xq_dma(c)
            for t in range(T0):
                if t % XCH == 0:
                    emit_xq_dma(t // XCH + XSLOTS - 1)
                emit_step(0, t)
            for l in (1, 2, 3):                  # tail flush
                while ready[l]:
                    emit_step(l, ready[l].pop(0))

            # ---- FC tail ----
            ps_fc = psp[0].tile([10, B], F32, tag="ps0", name="psfc")
            for j in range(W4):
                for k in range(POOL_K):
                    s = POOL_S * j + k
                    nc.tensor.matmul(ps_fc, lhsT=fc_sb[:, j, :],
                                     rhs=r[3][:, s * B:(s + 1) * B],
                                     start=(j == 0 and k == 0),
                                     stop=(j == W4 - 1 and k == POOL_K - 1),
                                     skip_group_check=True)
            out_sb = constp.tile([10, B], F32, tag="out_sb")
            nc.vector.tensor_scalar_add(out_sb, ps_fc, fcb_sb[:, 0:1])
            nc.sync.dma_start(out=out_d.ap(), in_=out_sb)

    nc.compile()
    return nc


def prep_in_maps(inputs, T0):
    """Host-side prep: shard x, transpose/scale weights. Returns per-core maps."""
    T, W4 = seq_lens(T0)
    f = lambda a: np.ascontiguousarray(np.asarray(a, dtype=np.float32))
    x = f(inputs["x"]).reshape(-1, T0)          # [512, T0]
    nb = x.shape[0] // B

    common = {}
    for l in range(4):
        wi = f(inputs[f"w_ih{l + 1}"])          # [H, I]
        wh = f(inputs[f"w_hh{l + 1}"])          # [H, H]
        bi = f(inputs[f"b_ih{l + 1}"]) + f(inputs[f"b_hh{l + 1}"])
        scale = 1.0 if l == 0 else (1.0 / POOL_K)
        common[f"wih{l}"] = np.ascontiguousarray((wi * scale).T).astype(np.float16)
        common[f"whh{l}"] = np.ascontiguousarray(wh.T).astype(np.float16)
        common[f"b{l}"] = np.ascontiguousarray(bi.reshape(-1, 1))  # [H, 1]
    common["fcw"] = np.ascontiguousarray((f(inputs["fc_w"]) / POOL_K).T).astype(np.float16)
    common["fcb"] = np.ascontiguousarray(f(inputs["fc_b"]).reshape(-1, 1))

    in_maps = []
    for c in range(nb):
        m = dict(common)
        xc = x[c * B:(c + 1) * B]               # [B, T0]
        m["xq"] = np.ascontiguousarray(xc.T).reshape(1, T0 * B).astype(np.float16)
        in_maps.append(m)
    return in_maps


_NC_CACHE = {}


def _install_ntff_hook():
    """Register the axon NTFF profile hook (the agent image's antenv lacks
    axon_hooks, so run_bass_kernel_spmd's trace path can't find it)."""
    import sys
    import types
    if "antenv.axon_hooks" in sys.modules:
        return
    mod = types.ModuleType("antenv.axon_hooks")
    mod._hook = None
    mod.set_axon_ntff_profile_hook = lambda h: setattr(mod, "_hook", h)
    mod.get_axon_ntff_profile_hook = lambda: mod._hook
    sys.modules["antenv.axon_hooks"] = mod
    try:
        import antenv
        antenv.axon_hooks = mod
    except ImportError:
        pass
    try:
        from trn_agent_boot.trn_boot import _ntff_profile_via_ctypes
        mod._hook = _ntff_profile_via_ctypes("/opt/axon/libaxon_pjrt.so")
    except Exception as e:  # degrade to no tracing
        print("ntff hook install failed:", e)


def run(inputs, T0=3437, core_ids=None, trace=False):
    if trace:
        _install_ntff_hook()
    if T0 not in _NC_CACHE:
        _NC_CACHE[T0] = build(T0)
    nc = _NC_CACHE[T0]
    in_maps = prep_in_maps(inputs, T0)
    if core_ids is None:
        core_ids = list(range(len(in_maps)))
    res = run_bass_kernel_spmd(nc, in_maps, core_ids=core_ids, trace=trace)
    out = np.concatenate([res.results[i]["out"].T for i in range(len(in_maps))],
                         axis=0).astype(np.float32)
    return out, res


def kernel(**inputs) -> np.ndarray:
    out, _ = run(inputs)
    return out
